# revision 52
# baseline (speedup 1.0000x reference)
"""Trainium2 Bass kernel for LoRA-fused QKV + RoPE + GQA causal attention + o_proj.

Problem (hardcoded): B=2, S=2048, H=2048, NH=16, KVH=4, HD=128, R=16.

Sharding: 8 cores = batch(2) x kv-head-group(4). Core c handles batch b=c//4,
kv head g=c%4 (q heads 4g..4g+3). Each core computes its 4 heads' attention and
a partial o_proj ([S,H] partial over its 512 o-dims); host sums 4 partials per
batch.

Key implementation choices (v2):
- LoRA and bias are folded host-side: w_eff = w + scale*(A@B)^T, bias applied
  via the scalar-engine Identity activation (per-partition bias operand) on the
  PSUM->SBUF evacuation. No on-device LoRA matmuls.
- All matmul operands are bf16 (fp32 PSUM accumulation). Everything on-device
  runs in "transposed space": projections produce qT/kT/vT [d, s] directly,
  scoresT [ks, qs] feeds AV without any on-device attn transpose.
- Softmax: no max-subtraction (scores are O(5)). exp on the scalar engine over
  PAIRS of score tiles (one [128, 2*CH] activation spanning 2 PSUM banks).
  Causal masking is applied post-exp by zeroing the upper triangle of diagonal
  tiles in-place with gpsimd affine_select. Denominators: DVE pairwise adds of
  the bf16 attn tiles, then one ones[128,128]-stationary matmul per head that
  both column-sums and broadcasts to all partitions; reciprocal_approx_fast on
  the full tile; one DVE multiply normalizes.
- o_proj matmuls are interleaved into the NEXT chunk's attention pair loop so
  the PE never idles while the scalar engine computes exp.
- Weights/cos/sin are loaded once (bf16 fits in SBUF); x double-buffered; DMAs
  spread across the sync/gpsimd/scalar queues. Output partials are written as
  bf16 and summed on host in fp32.
"""

import hashlib
import numpy as np
import ml_dtypes

import concourse.bass as bass
import concourse.mybir as mybir
import concourse.tile as tile
from concourse import bacc
from concourse.bass_utils import run_bass_kernel_spmd

B, S, H = 2, 2048, 2048
NH, KVH, HD = 16, 4, 128
R = 16
LORA_SCALE = 32.0 / 16.0
ATTN_SCALE = HD ** -0.5

NCORES = 8
GQ = NH // KVH          # 4 q heads per core
NT = GQ + 2             # 6 projection tiles: 4 q heads, 1 k, 1 v
QD = GQ * HD            # 512
CH = 512                # s-chunk width
NCH = S // CH           # 4 s-chunks
KT = H // 128           # 16 contraction k-tiles
NKS = S // 128          # 16 ks tiles
F32 = mybir.dt.float32
BF16 = mybir.dt.bfloat16
BF16_NP = np.dtype(ml_dtypes.bfloat16)

# tile classification codes (host-computed from exp(mask) tiles)
SKIP, PLAIN, MASKED = 0, 1, 2

# content tag: force a fresh NEFF cache key whenever this file changes
# (the jax/neuron compile cache does not key on the embedded BIR)
with open(__file__, "rb") as _f:
    KTAG = hashlib.sha1(_f.read()).hexdigest()[:10]
K_TAG_INT = int(KTAG, 16)


def _build(cls_grid, causal):
    """Build the SPMD program. cls_grid[i][j] in {SKIP, PLAIN, MASKED} for
    scoresT tile (ks_tile i, qs_chunk j). causal=True zeroes diagonal tiles on
    device with affine_select (no emaskT input)."""
    nc = bacc.Bacc("TRN2", target_bir_lowering=False)

    # host-packed for contiguous per-partition DMA:
    # x_pre[c, p, kt, s'] = x[b][s = c*CH+s', h = kt*128+p]
    xT = nc.dram_tensor("xT", [NCH, 128, KT, CH], BF16, kind="ExternalInput")
    # w_pre[p, t, kt, o] = w_eff[t*128+o, kt*128+p]  (LoRA folded in)
    wT = nc.dram_tensor("wT", [128, NT, KT, 128], BF16, kind="ExternalInput")
    # bias_pre[p, t] = bias[t*128+p]
    biasT = nc.dram_tensor("biasT", [128, NT], F32, kind="ExternalInput")
    # cache-buster: the PJRT NEFF cache hashes the HLO minus backend_config
    # (where the BIR rides); a tag-dependent input SHAPE forces a new hash.
    DL = (K_TAG_INT % 97) + 1
    dummy = nc.dram_tensor("cachetag", [1, DL], F32, kind="ExternalInput")
    cosT = nc.dram_tensor("cosT", [HD, S], BF16, kind="ExternalInput")
    ssT = nc.dram_tensor("ssT", [HD, S], BF16, kind="ExternalInput")
    any_masked = any(cls_grid[i][j] == MASKED for i in range(NKS) for j in range(NCH))
    emaskT = None
    if not causal and any_masked:
        emaskT = nc.dram_tensor("emaskT", [S, S], BF16, kind="ExternalInput")
    # ow_pre[p, g, n] = o_w[n, g*128+p  (+ QD*core_g)]
    owT = nc.dram_tensor("owT", [128, GQ, H], BF16, kind="ExternalInput")
    out_p = nc.dram_tensor("out_p", [S, H], BF16, kind="ExternalOutput")

    live_per_j = [[i for i in range(NKS) if cls_grid[i][jj] != SKIP]
                  for jj in range(NCH)]
    masked_per_j = [[i for i in range(NKS) if cls_grid[i][jj] == MASKED]
                    for jj in range(NCH)]
    # per-chunk prerequisite: attention(j) can only run after the proj chunk
    # that produces its highest live k/v tile (and its own q chunk).
    need = [max(jj, max(live_per_j[jj]) // (CH // 128)) for jj in range(NCH)]
    QCH_BUFS = max(2, max(need[jj] - jj for jj in range(NCH)) + 1)

    with tile.TileContext(nc) as tc:
        from concourse.masks import make_identity
        with tc.tile_pool(name="consts", bufs=1) as consts, \
             tc.tile_pool(name="persist", bufs=1) as persist, \
             tc.tile_pool(name="qch", bufs=QCH_BUFS) as qch_pool, \
             tc.tile_pool(name="outp", bufs=2) as outp_pool, \
             tc.tile_pool(name="p1", bufs=3) as p1, \
             tc.tile_pool(name="p1s", bufs=3) as p1s, \
             tc.tile_pool(name="attnp", bufs=3) as attnp, \
             tc.tile_pool(name="accp", bufs=2) as accp, \
             tc.tile_pool(name="tmpp", bufs=2) as tmpp, \
             tc.tile_pool(name="p2", bufs=4) as p2, \
             tc.tile_pool(name="stgp", bufs=4) as stgp, \
             tc.tile_pool(name="fin", bufs=2) as fin, \
             tc.tile_pool(name="pp_qk", bufs=2, space="PSUM") as pp_qk, \
             tc.tile_pool(name="pp_o", bufs=1, space="PSUM") as pp_o, \
             tc.tile_pool(name="pp_p", bufs=2, space="PSUM") as pp_p, \
             tc.tile_pool(name="pp_den", bufs=1, space="PSUM") as pp_den:

            # round-robin DMA queue choice (keep vector/tensor queues free)
            _q = [nc.sync, nc.gpsimd, nc.scalar]
            _qi = [0]

            def dq():
                e = _q[_qi[0] % 3]
                _qi[0] += 1
                return e

            # out_p DMAs avoid the gpsimd queue: the causal-masking
            # affine_selects live there and sit on the exp->select->AV
            # critical path during attention
            _qo = [0]

            def dq_out():
                e = [nc.sync, nc.scalar][_qo[0] % 2]
                _qo[0] += 1
                return e

            # ---- persistent tiles ----
            kT_full = persist.tile([128, S], BF16, tag="kT_full")
            v_nat = persist.tile([128, NKS, 128], BF16, tag="v_nat")  # [ks, tile, d]
            ow_sb = persist.tile([128, GQ, H], BF16, tag="ow_sb")
            w_all = persist.tile([128, NT, KT, 128], BF16, tag="w_all")
            cos_full = persist.tile([128, S], BF16, tag="cos_full")
            ss_full = persist.tile([128, S], BF16, tag="ss_full")
            bias_sb = consts.tile([128, NT], F32, tag="bias_sb")

            q_chunks = {}

            def prefetch_x(c):
                if c >= NCH or c in x_tiles:
                    return
                x_c = p1s.tile([128, KT, CH], BF16, tag="x_c", name=f"x_{c}")
                for kq in range(4):
                    dq().dma_start(out=x_c[:, bass.ds(kq * 4, 4), :],
                                   in_=xT[c, :, bass.ds(kq * 4, 4), :])
                x_tiles[c] = x_c

            # ---- first-chunk-critical DMAs: just-in-time ordering ----
            # DMA issue order == transfer start order per queue, and the HBM
            # path is bandwidth-shared, so the pieces the first proj groups
            # consume must be issued FIRST; cos/ss (needed ~4us in) and ow
            # (needed ~40us in) go behind them.
            x_c0 = p1s.tile([128, KT, CH], BF16, tag="x_c", name="x_0")
            x_tiles = {0: x_c0}
            # deal the chunk-0 pieces to the 3 queues in CONSUMPTION order:
            # group t consumes all 16 x k-tiles plus its w halves; the k
            # group's second w half is needed ~1.7us in, so it must not sit
            # behind the whole x load. w halves (8 k-tiles) let each group
            # start after its first half lands; x starts with 1-kt pieces.
            def _xp(a, b):
                def f(eng):
                    eng.dma_start(out=x_c0[:, a:b, :], in_=xT[0, :, a:b, :])
                return f

            def _wp(t, k0):
                def f(eng):
                    eng.dma_start(out=w_all[:, t, k0:k0 + 8, :],
                                  in_=wT[:, t, k0:k0 + 8, :])
                return f

            _startup = [
                lambda eng: eng.dma_start(out=w_all[:, GQ, 0:4, :],
                                          in_=wT[:, GQ, 0:4, :]),
                _xp(0, 1), _xp(1, 2),
                lambda eng: eng.dma_start(out=w_all[:, GQ, 4:8, :],
                                          in_=wT[:, GQ, 4:8, :]),
                _xp(2, 4), _wp(GQ, 8), _xp(4, 6), _xp(6, 8),
                _wp(GQ + 1, 0), _xp(8, 10), _wp(GQ + 1, 8), _xp(10, 12),
                _wp(0, 0), _xp(12, 14), _wp(0, 8), _xp(14, 16),
                _wp(1, 0), _wp(1, 8), _wp(2, 0), _wp(2, 8),
                _wp(3, 0), _wp(3, 8),
            ]
            # bias is tiny but needed by the first evacuation (~3.5us in)
            _startup.insert(4, lambda eng: eng.dma_start(out=bias_sb,
                                                         in_=biasT[:, :]))
            for si, f in enumerate(_startup):
                f([nc.sync, nc.gpsimd, nc.scalar][si % 3])
            nc.scalar.dma_start(out=ss_full, in_=ssT[:, :])
            nc.gpsimd.dma_start(out=cos_full, in_=cosT[:, :])
            dummy_sb = consts.tile([1, 128], F32, tag="dummy_sb")
            nc.sync.dma_start(out=dummy_sb[:, 0:DL], in_=dummy[:, :])
            nc.gpsimd.dma_start(out=ow_sb, in_=owT[:, :, :])

            # warm the scalar-engine exp table (emitted after the critical
            # startup DMAs so its ACT_TABLE_LOAD doesn't block the scalar
            # queue; it only must precede attention(0)'s first real exp)
            warm = consts.tile([1, 2], F32, tag="warm")
            nc.vector.memset(warm, 0.0)
            warm2 = consts.tile([1, 2], F32, tag="warm2")
            nc.scalar.activation(out=warm2, in_=warm,
                                 func=mybir.ActivationFunctionType.Exp)

            # ---- small constants (engine ops, no DMA) ----
            ones_mat = consts.tile([128, 128], BF16, tag="ones_mat")
            nc.vector.memset(ones_mat, 1.0)
            ident = consts.tile([128, 128], BF16, tag="ident")
            make_identity(nc, ident)

            pending_oproj = [None]
            deferred_fin = []   # up to 2 of (finalize_fn, h, ps_o, acc); spans phases
            _evac = [0]
            _oq = [0]

            def make_oproj_items(args, final=False):
                """Return a list of closures, each emitting one o_proj matmul;
                the 4th of each group also emits the evacuation + out DMA."""
                cc, outT_ch = args
                items = []
                for st4 in range(CH // 128):
                    ssl = bass.ds(st4 * 128, 128)
                    dsl = bass.ds((cc * (CH // 128) + st4) * 128, 128)
                    for nch in range(NCH):
                        nsl = bass.ds(nch * CH, CH)

                        def mk(st4=st4, nch=nch, ssl=ssl, dsl=dsl, nsl=nsl):
                            state = {}

                            def first():
                                state["ps3"] = pp_p.tile([128, CH], F32, tag="p",
                                                         name="ps3")
                                nc.tensor.matmul(state["ps3"], outT_ch[0][:, ssl],
                                                 ow_sb[:, 0, nsl],
                                                 start=True, stop=False)

                            def mid(h):
                                def f():
                                    nc.tensor.matmul(state["ps3"],
                                                     outT_ch[h][:, ssl],
                                                     ow_sb[:, h, nsl],
                                                     start=False, stop=(h == GQ - 1))
                                    if h == GQ - 1:
                                        stg = stgp.tile([128, CH], BF16, tag="stg")
                                        # 1:3 ACT:DVE — attention phases are
                                        # exp-paced on ACT; DVE has headroom
                                        if _evac[0] % 4 == 0:
                                            nc.scalar.activation(
                                                out=stg, in_=state["ps3"],
                                                func=mybir.ActivationFunctionType.Copy)
                                        else:
                                            nc.vector.tensor_copy(out=stg,
                                                                  in_=state["ps3"])
                                        _evac[0] += 1
                                        if final:
                                            # split across two queues: halves
                                            # the last tiles' transfer latency
                                            half = CH // 2
                                            n0 = nch * CH
                                            dq().dma_start(
                                                out=out_p[dsl, bass.ds(n0, half)],
                                                in_=stg[:, 0:half])
                                            dq().dma_start(
                                                out=out_p[dsl, bass.ds(n0 + half, half)],
                                                in_=stg[:, half:CH])
                                        else:
                                            dq_out().dma_start(
                                                out=out_p[dsl, nsl], in_=stg)
                                return f

                            return [first] + [mid(h) for h in range(1, GQ)]

                        items.extend(mk())
                return items

            def build_proj_groups(c):
                """Projection matmuls for chunk c as one closure per weight
                group, ordered [k, q0, v, q1, q2, q3]: the first three run
                before attention(c); q1..q3 weave into the attention head loop
                (head h's exp time covers head h+1's projection group)."""
                sl = bass.ds(c * CH, CH)
                x_c = x_tiles[c]
                q_ch = [qch_pool.tile([128, CH], BF16, tag=f"qch{h}",
                                      name=f"qch{h}_{c}")
                        for h in range(GQ)]
                q_chunks[c] = q_ch

                def mk_group(t):
                    def f():
                        ps_p = pp_p.tile([128, CH], F32, tag="p", name="ps_p")
                        for kt in range(KT):
                            nc.tensor.matmul(ps_p, w_all[:, t, kt, :],
                                             x_c[:, kt, :],
                                             start=(kt == 0), stop=(kt == KT - 1))
                        raw = p1.tile([128, CH], BF16, tag="raw",
                                      name=f"raw_{c}_{t}")
                        nc.scalar.activation(
                            out=raw, in_=ps_p,
                            func=mybir.ActivationFunctionType.Identity,
                            bias=bias_sb[:, bass.ds(t, 1)])
                        if t == NT - 1:   # v: no rope; transpose to v_nat
                            for i4 in range(CH // 128):
                                i = c * (CH // 128) + i4
                                ps_t = pp_p.tile([128, 128], BF16, tag="p",
                                                 name="ps_t")
                                nc.tensor.transpose(
                                    ps_t, raw[:, bass.ds(i4 * 128, 128)], ident)
                                nc.vector.tensor_copy(out=v_nat[:, i, :],
                                                      in_=ps_t)
                        else:
                            # rope: dst = raw*cos + swap(raw)*ss
                            sw = p1.tile([128, CH], BF16, tag="sw",
                                         name=f"sw_{c}_{t}")
                            dq().dma_start(out=sw[0:64, :], in_=raw[64:128, :])
                            dq().dma_start(out=sw[64:128, :], in_=raw[0:64, :])
                            nc.vector.tensor_mul(sw, sw, ss_full[:, sl])
                            dst = q_ch[t] if t < GQ else kT_full[:, sl]
                            nc.vector.tensor_mul(dst, raw, cos_full[:, sl])
                            nc.vector.tensor_add(dst, dst, sw)
                    return f

                return [mk_group(t) for t in [GQ, GQ + 1] + list(range(GQ))]

            def emit_attention(j, oproj_items, qgroups=()):
                """Attention for qs-chunk j; interleaves pending o_proj matmul
                items (from the previous chunk) into the pair loop."""
                sl = bass.ds(j * CH, CH)
                live = live_per_j[j]
                masked = set(masked_per_j[j])
                # pair up live tiles; odd count -> trailing singleton
                pairs = [(live[2 * m], live[2 * m + 1] if 2 * m + 1 < len(live)
                          else None) for m in range((len(live) + 1) // 2)]
                npairs = len(pairs)
                oi = [0]
                total_slots = GQ * (npairs + 1)
                slot = [0]

                def pull_oproj():
                    if not oproj_items:
                        return
                    remaining = len(oproj_items) - oi[0]
                    slots_left = total_slots - slot[0]
                    k = -(-remaining // max(1, slots_left))  # ceil
                    for _ in range(k):
                        if oi[0] < len(oproj_items):
                            oproj_items[oi[0]]()
                            oi[0] += 1
                    slot[0] += 1

                outT_ch = [outp_pool.tile([128, CH], BF16, tag=f"outT{h}",
                                          name=f"outT{h}_{j}") for h in range(GQ)]
                q_ch = q_chunks[j]

                def finalize(h, ps_o, acc):
                    # ones[128,128] stationary: column sums broadcast to all
                    # partitions in one matmul; then fast reciprocal + multiply.
                    ps_den = pp_den.tile([128, CH], F32, tag="den", name="ps_den")
                    nc.tensor.matmul(ps_den, ones_mat, acc, start=True, stop=True)
                    recip = fin.tile([128, CH], F32, tag="recip")
                    nc.vector.reciprocal_approx_fast(out=recip, in_=ps_den)
                    nc.vector.tensor_mul(outT_ch[h], ps_o, recip)

                for h in range(GQ):
                    qh = q_ch[h]
                    acc = accp.tile([128, CH], BF16, tag="acc", name=f"acc_{j}_{h}")
                    n_mm = sum(1 if i1 is None else 2 for i0, i1 in pairs)
                    attns = {}

                    def off_of(i):
                        # causal diagonal tile at offset d = i-4j: columns
                        # q < 128*d are fully masked -> clip them everywhere
                        if causal and i is not None and i in masked:
                            return 128 * (i - 4 * j)
                        return 0

                    def emit_qk_exp(n, h=h, qh=qh):
                        i0, i1 = pairs[n]
                        o0, o1 = off_of(i0), off_of(i1)
                        qk = pp_qk.tile([128, 2, CH], F32, tag="qk", name="qk")
                        nc.tensor.matmul(qk[:, 0, o0:],
                                         kT_full[:, bass.ds(i0 * 128, 128)],
                                         qh[:, o0:], start=True, stop=True)
                        if i1 is not None:
                            nc.tensor.matmul(qk[:, 1, o1:],
                                             kT_full[:, bass.ds(i1 * 128, 128)],
                                             qh[:, o1:], start=True, stop=True)
                        attn = attnp.tile([128, 2, CH], BF16, tag="attn",
                                          name="attn")
                        if i1 is not None and o0 == 0 and o1 <= 128:
                            # one paired exp is cheaper than two split ones for
                            # small clips; the clipped columns get exp(stale)
                            # which nothing consumes
                            nc.scalar.activation(out=attn, in_=qk,
                                                 func=mybir.ActivationFunctionType.Exp,
                                                 scale=float(ATTN_SCALE))
                        else:
                            for k, (i, o) in enumerate(((i0, o0), (i1, o1))):
                                if i is None:
                                    continue
                                nc.scalar.activation(out=attn[:, k, o:],
                                                     in_=qk[:, k, o:],
                                                     func=mybir.ActivationFunctionType.Exp,
                                                     scale=float(ATTN_SCALE))
                        for k, (i, o) in enumerate(((i0, o0), (i1, o1))):
                            if i is None or i not in masked:
                                continue
                            if causal:
                                # zero the partially-masked triangle: keep
                                # where q' >= p (q' relative to the clipped
                                # slice start 128*d); is_gt is the only ALU op
                                # the compiler implements for affine_select
                                nc.gpsimd.affine_select(
                                    out=attn[:, k, o:], in_=attn[:, k, o:],
                                    compare_op=mybir.AluOpType.is_gt,
                                    fill=0.0,
                                    base=1,
                                    channel_multiplier=-1,
                                    pattern=[[1, CH - o]],
                                )
                            else:
                                mt = p2.tile([128, CH], BF16, tag="m_tile",
                                             name=f"mt_{j}_{h}_{i}")
                                dq().dma_start(out=mt,
                                               in_=emaskT[bass.ds(i * 128, 128), sl])
                                nc.vector.tensor_mul(attn[:, k, :], attn[:, k, :], mt)
                        attns[n] = attn

                    # lookahead: 2 score-pair tiles in flight before the first
                    # AV; the previous head's finalize (PE ps_den matmul) must
                    # be emitted BEFORE this head's first AV (ps_o bufs=1).
                    emit_qk_exp(0)
                    if npairs > 1:
                        emit_qk_exp(1)
                    if h < len(qgroups):
                        # next head's projection group: its PE time is covered
                        # by this head's exp backlog on the scalar engine
                        qgroups[h]()
                    # flush deferred finalizes BEFORE pulling o_proj items:
                    # at a phase boundary those items read the outT tiles the
                    # finalizes write (emission order defines the dependency).
                    # ps_o has 2 banks, so each finalize can ride 2 heads
                    # behind its accumulation — the den matmul then never
                    # waits on the DVE chain.
                    while deferred_fin:
                        fn, fh, fo, fa = deferred_fin.pop(0)
                        fn(fh, fo, fa)
                    pull_oproj()
                    ps_o = pp_o.tile([128, CH], F32, tag="o", name="ps_o")
                    mm_done = 0
                    for n in range(npairs):
                        if n + 2 < npairs:
                            emit_qk_exp(n + 2)
                        i0, i1 = pairs[n]
                        o0, o1 = off_of(i0), off_of(i1)
                        attn = attns.pop(n)
                        # denominator accumulation on DVE (bf16, 2x mode).
                        # clipped (fully-masked) columns contribute zero and
                        # are skipped; the first live tile of any chunk always
                        # covers the full width, so acc is fully initialized.
                        if n == 0:
                            assert o0 == 0
                            if i1 is None:
                                nc.vector.tensor_copy(out=acc, in_=attn[:, 0, :])
                            elif o1 == 0:
                                nc.vector.tensor_add(acc, attn[:, 0, :], attn[:, 1, :])
                            else:
                                nc.vector.tensor_copy(out=acc, in_=attn[:, 0, :])
                                nc.vector.tensor_add(acc[:, o1:], acc[:, o1:],
                                                     attn[:, 1, o1:])
                        elif i1 is not None and o0 == 0 and o1 == 0:
                            tmp = tmpp.tile([128, CH], BF16, tag="tmp")
                            nc.vector.tensor_add(tmp, attn[:, 0, :], attn[:, 1, :])
                            nc.vector.tensor_add(acc, acc, tmp)
                        else:
                            for k, (i, o) in enumerate(((i0, o0), (i1, o1))):
                                if i is None:
                                    continue
                                nc.vector.tensor_add(acc[:, o:], acc[:, o:],
                                                     attn[:, k, o:])
                        # AV accumulation (clipped to live columns)
                        for k, (i, o) in enumerate(((i0, o0), (i1, o1))):
                            if i is None:
                                continue
                            nc.tensor.matmul(ps_o[:, o:], v_nat[:, i, :],
                                             attn[:, k, o:],
                                             start=(mm_done == 0),
                                             stop=(mm_done == n_mm - 1))
                            mm_done += 1
                        pull_oproj()
                    # the trailing finalizes deliberately span into the NEXT
                    # phase: their den-matmuls would otherwise stall the PE
                    # queue on the DVE accumulation chain at the boundary
                    deferred_fin.append((finalize, h, ps_o, acc))
                # drain any leftover o_proj items
                while oproj_items and oi[0] < len(oproj_items):
                    oproj_items[oi[0]]()
                    oi[0] += 1
                return outT_ch

            projected = set()

            def ensure_proj(c):
                if c >= NCH or c in projected:
                    return
                projected.add(c)
                prefetch_x(c)
                for g in build_proj_groups(c):
                    g()

            # NOTE: weaving projection matmuls into the attention pair loop
            # (tried at both item and group granularity) consistently REGRESSED
            # ~10-40us: the tighter cross-engine coupling costs ~60ns of
            # semaphore wait per matmul, exceeding the overlap gain. Keep the
            # projection phases sequential; only o_proj interleaves.
            built = {}

            def groups_for(c):
                if c not in built:
                    prefetch_x(c)
                    built[c] = build_proj_groups(c)
                return built[c]

            for c in range(NCH):
                ensure_proj(c)
                for j in range(NCH):
                    if need[j] == c:
                        oproj_items = (make_oproj_items(pending_oproj[0])
                                       if pending_oproj[0] is not None else [])
                        prefetch_x(c + 1)
                        outT = emit_attention(j, oproj_items)
                        pending_oproj[0] = (j, outT)

            while deferred_fin:
                fn, fh, fo, fa = deferred_fin.pop(0)
                fn(fh, fo, fa)
            if pending_oproj[0] is not None:
                for it in make_oproj_items(pending_oproj[0], final=True):
                    it()

    nc.finalize()
    return nc


_cache = {}


def _get_program(key, cls_grid, causal):
    if key not in _cache:
        _cache[key] = _build(cls_grid, causal)
    return _cache[key]


def _classify(em_t):
    """em_t: exp(mask).T [S, S] (ks, qs). Returns tuple-of-tuples class grid
    [NKS][NCH]."""
    grid = []
    for i in range(NKS):
        row = []
        for j in range(NCH):
            t = em_t[i * 128:(i + 1) * 128, j * CH:(j + 1) * CH]
            mx = t.max()
            mn = t.min()
            if mx == 0.0:
                row.append(SKIP)
            elif mn == 1.0 and mx == 1.0:
                row.append(PLAIN)
            else:
                row.append(MASKED)
        grid.append(tuple(row))
    return tuple(grid)


def _causal_grid():
    g = []
    for i in range(NKS):
        row = []
        for j in range(NCH):
            if i >= 4 * j + 4:
                row.append(SKIP)
            elif i >= 4 * j:
                row.append(MASKED)
            else:
                row.append(PLAIN)
        g.append(tuple(row))
    return tuple(g)


def _is_exact_causal(emaskT_b):
    """True iff exp(mask).T's diagonal band is exactly the causal 0/1
    pattern (off-band is covered by the grid comparison)."""
    p = np.arange(128)[:, None]
    for jj in range(NCH):
        for i in range(4 * jj, 4 * jj + 4):
            t = emaskT_b[i * 128:(i + 1) * 128, jj * CH:(jj + 1) * CH]
            d = i - 4 * jj
            q = np.arange(CH)[None, :]
            want = (p - q + 128 * d <= 0).astype(np.float32)
            if not np.array_equal(t, want):
                return False
    return True


def kernel(hidden_states, cos, sin, attention_mask,
           q_w, k_w, v_w, q_b, k_b, v_b,
           q_A, q_B, k_A, k_B, v_A, v_B, o_w):
    f32 = np.float32
    hidden_states = np.ascontiguousarray(hidden_states, dtype=f32)
    cos = np.asarray(cos, dtype=f32)
    sin = np.asarray(sin, dtype=f32)
    mask = np.asarray(attention_mask, dtype=f32)[:, 0]  # [B, S, S]

    # host-side shared prep
    with np.errstate(under="ignore", over="ignore"):
        emask = np.exp(np.minimum(mask, 80.0))  # [B, S, S]; clamp avoids inf
    emaskT = [np.ascontiguousarray(emask[b].T) for b in range(B)]
    grids = [_classify(emaskT[b]) for b in range(B)]
    if grids[0] != grids[1]:
        # classifications must agree across cores (same SPMD program):
        # degrade to "multiply everywhere except both-skip"
        grid = tuple(tuple(MASKED if (grids[0][i][j] != SKIP or grids[1][i][j] != SKIP)
                           else SKIP for j in range(NCH)) for i in range(NKS))
    else:
        grid = grids[0]
    # every qs column needs at least one live tile (else div by zero);
    # fall back to fully dense+masked if any column is empty
    for j in range(NCH):
        if all(grid[i][j] == SKIP for i in range(NKS)):
            grid = tuple(tuple(MASKED for _ in range(NCH)) for _ in range(NKS))
            break

    causal = (grid == _causal_grid()
              and all(_is_exact_causal(emaskT[b]) for b in range(B)))

    nc = _get_program((grid, causal), grid, causal)

    # x_pre[c, p, kt, s'] = x[b][c*CH+s', kt*128+p]
    xT = [np.ascontiguousarray(
        hidden_states[b].reshape(NCH, CH, KT, 128).transpose(0, 3, 2, 1)).astype(BF16_NP)
        for b in range(B)]
    cosT = [np.ascontiguousarray(cos[b].T).astype(BF16_NP) for b in range(B)]
    ss = np.concatenate([-sin[:, :, :HD // 2], sin[:, :, HD // 2:]], axis=-1)  # [B,S,HD]
    ssT = [np.ascontiguousarray(ss[b].T).astype(BF16_NP) for b in range(B)]

    # fold LoRA into the base weights (exact same math, done in fp32 on host)
    q_lora = LORA_SCALE * (q_A @ q_B).T   # [q_dim, H]
    k_lora = LORA_SCALE * (k_A @ k_B).T   # [kv_dim, H]
    v_lora = LORA_SCALE * (v_A @ v_B).T

    in_maps = []
    for c in range(NCORES):
        b, g = divmod(c, KVH)
        qsl = slice(QD * g, QD * (g + 1))
        ksl = slice(HD * g, HD * (g + 1))
        w_cat = np.concatenate([q_w[qsl] + q_lora[qsl],
                                k_w[ksl] + k_lora[ksl],
                                v_w[ksl] + v_lora[ksl]], axis=0)  # [768, H]
        # w_pre[p, t, kt, o] = w_cat[t*128+o, kt*128+p]
        wT_c = w_cat.reshape(NT, 128, KT, 128).transpose(3, 0, 2, 1)
        bias_c = np.concatenate([q_b[qsl], k_b[ksl], v_b[ksl]])  # [768]
        biasT_c = np.ascontiguousarray(bias_c.reshape(NT, 128).T, dtype=f32)
        owT_c = np.ascontiguousarray(
            o_w[:, qsl].T.reshape(GQ, 128, H).transpose(1, 0, 2)).astype(BF16_NP)
        m = {
            "xT": xT[b],
            "wT": np.ascontiguousarray(wT_c).astype(BF16_NP),
            "biasT": biasT_c,
            "cachetag": np.zeros((1, (K_TAG_INT % 97) + 1), f32),
            "cosT": cosT[b],
            "ssT": ssT[b],
            "owT": owT_c,
        }
        if not causal and any(grid[i][j] == MASKED for i in range(NKS) for j in range(NCH)):
            m["emaskT"] = emaskT[b].astype(BF16_NP)
        in_maps.append(m)

    res = run_bass_kernel_spmd(nc, in_maps, core_ids=list(range(NCORES)))
    outs = [np.asarray(r["out_p"]).astype(f32) for r in res.results]
    full = np.empty((B, S, H), f32)
    for b in range(B):
        full[b] = outs[KVH * b]
        for g in range(1, KVH):
            full[b] += outs[KVH * b + g]
    return full


# revision 53
# speedup vs baseline: 1.1713x; 1.1713x over previous
"""Trainium2 Bass kernel for LoRA-fused QKV + RoPE + GQA causal attention + o_proj.

Problem (hardcoded): B=2, S=2048, H=2048, NH=16, KVH=4, HD=128, R=16.

Sharding: 8 cores = batch(2) x kv-head-group(4). Core c handles batch b=c//4,
kv head g=c%4 (q heads 4g..4g+3). Each core computes its 4 heads' attention and
a partial o_proj ([S,H] partial over its 512 o-dims); host sums 4 partials per
batch.

Key implementation choices (v2):
- LoRA and bias are folded host-side: w_eff = w + scale*(A@B)^T, bias applied
  via the scalar-engine Identity activation (per-partition bias operand) on the
  PSUM->SBUF evacuation. No on-device LoRA matmuls.
- All matmul operands are bf16 (fp32 PSUM accumulation). Everything on-device
  runs in "transposed space": projections produce qT/kT/vT [d, s] directly,
  scoresT [ks, qs] feeds AV without any on-device attn transpose.
- Softmax: no max-subtraction (scores are O(5)). exp on the scalar engine over
  PAIRS of score tiles (one [128, 2*CH] activation spanning 2 PSUM banks).
  Causal masking is applied post-exp by zeroing the upper triangle of diagonal
  tiles in-place with gpsimd affine_select. Denominators: DVE pairwise adds of
  the bf16 attn tiles, then one ones[128,128]-stationary matmul per head that
  both column-sums and broadcasts to all partitions; reciprocal_approx_fast on
  the full tile; one DVE multiply normalizes.
- o_proj matmuls are interleaved into the NEXT chunk's attention pair loop so
  the PE never idles while the scalar engine computes exp.
- Weights/cos/sin are loaded once (bf16 fits in SBUF); x double-buffered; DMAs
  spread across the sync/gpsimd/scalar queues. Output partials are written as
  bf16 and summed on host in fp32.
"""

import hashlib
import numpy as np
import ml_dtypes

import concourse.bass as bass
import concourse.mybir as mybir
import concourse.tile as tile
from concourse import bacc
from concourse.bass_utils import run_bass_kernel_spmd

B, S, H = 2, 2048, 2048
NH, KVH, HD = 16, 4, 128
R = 16
LORA_SCALE = 32.0 / 16.0
ATTN_SCALE = HD ** -0.5

NCORES = 8
GQ = NH // KVH          # 4 q heads per core
NT = GQ + 2             # 6 projection tiles: 4 q heads, 1 k, 1 v
QD = GQ * HD            # 512
CH = 512                # s-chunk width
NCH = S // CH           # 4 s-chunks
KT = H // 128           # 16 contraction k-tiles
NKS = S // 128          # 16 ks tiles
F32 = mybir.dt.float32
BF16 = mybir.dt.bfloat16
BF16_NP = np.dtype(ml_dtypes.bfloat16)

# tile classification codes (host-computed from exp(mask) tiles)
SKIP, PLAIN, MASKED = 0, 1, 2

# content tag: force a fresh NEFF cache key whenever this file changes
# (the jax/neuron compile cache does not key on the embedded BIR)
with open(__file__, "rb") as _f:
    KTAG = hashlib.sha1(_f.read()).hexdigest()[:10]
K_TAG_INT = int(KTAG, 16)


def _build(cls_grid, causal):
    """Build the SPMD program. cls_grid[i][j] in {SKIP, PLAIN, MASKED} for
    scoresT tile (ks_tile i, qs_chunk j). causal=True zeroes diagonal tiles on
    device with affine_select (no emaskT input)."""
    nc = bacc.Bacc("TRN2", target_bir_lowering=False)

    # host-packed for contiguous per-partition DMA:
    # x_pre[c, p, kt, s'] = x[b][s = c*CH+s', h = kt*128+p]
    xT = nc.dram_tensor("xT", [NCH, 128, KT, CH], BF16, kind="ExternalInput")
    # w_pre[p, t, kt, o] = w_eff[t*128+o, kt*128+p]  (LoRA folded in)
    wT = nc.dram_tensor("wT", [128, NT, KT, 128], BF16, kind="ExternalInput")
    # bias_pre[p, t] = bias[t*128+p]
    biasT = nc.dram_tensor("biasT", [128, NT], F32, kind="ExternalInput")
    # cache-buster: the PJRT NEFF cache hashes the HLO minus backend_config
    # (where the BIR rides); a tag-dependent input SHAPE forces a new hash.
    DL = (K_TAG_INT % 97) + 1
    dummy = nc.dram_tensor("cachetag", [1, DL], F32, kind="ExternalInput")
    cosT = nc.dram_tensor("cosT", [HD, S], BF16, kind="ExternalInput")
    ssT = nc.dram_tensor("ssT", [HD, S], BF16, kind="ExternalInput")
    any_masked = any(cls_grid[i][j] == MASKED for i in range(NKS) for j in range(NCH))
    emaskT = None
    if not causal and any_masked:
        emaskT = nc.dram_tensor("emaskT", [S, S], BF16, kind="ExternalInput")
    # ow_pre[p, g, n] = o_w[n, g*128+p  (+ QD*core_g)]
    owT = nc.dram_tensor("owT", [128, GQ, H], BF16, kind="ExternalInput")
    out_p = nc.dram_tensor("out_p", [S, H], BF16, kind="ExternalOutput")

    live_per_j = [[i for i in range(NKS) if cls_grid[i][jj] != SKIP]
                  for jj in range(NCH)]
    masked_per_j = [[i for i in range(NKS) if cls_grid[i][jj] == MASKED]
                    for jj in range(NCH)]
    # per-chunk prerequisite: attention(j) can only run after the proj chunk
    # that produces its highest live k/v tile (and its own q chunk).
    need = [max(jj, max(live_per_j[jj]) // (CH // 128)) for jj in range(NCH)]
    QCH_BUFS = max(2, max(need[jj] - jj for jj in range(NCH)) + 1)

    with tile.TileContext(nc) as tc:
        from concourse.masks import make_identity
        with tc.tile_pool(name="consts", bufs=1) as consts, \
             tc.tile_pool(name="persist", bufs=1) as persist, \
             tc.tile_pool(name="qch", bufs=QCH_BUFS) as qch_pool, \
             tc.tile_pool(name="outp", bufs=2) as outp_pool, \
             tc.tile_pool(name="p1", bufs=3) as p1, \
             tc.tile_pool(name="p1s", bufs=3) as p1s, \
             tc.tile_pool(name="attnp", bufs=3) as attnp, \
             tc.tile_pool(name="accp", bufs=2) as accp, \
             tc.tile_pool(name="tmpp", bufs=2) as tmpp, \
             tc.tile_pool(name="p2", bufs=4) as p2, \
             tc.tile_pool(name="stgp", bufs=4) as stgp, \
             tc.tile_pool(name="fin", bufs=2) as fin, \
             tc.tile_pool(name="pp_qk", bufs=2, space="PSUM") as pp_qk, \
             tc.tile_pool(name="pp_o", bufs=1, space="PSUM") as pp_o, \
             tc.tile_pool(name="pp_p", bufs=2, space="PSUM") as pp_p, \
             tc.tile_pool(name="pp_den", bufs=1, space="PSUM") as pp_den:

            # round-robin DMA queue choice (keep vector/tensor queues free)
            _q = [nc.sync, nc.gpsimd, nc.scalar]
            _qi = [0]

            def dq():
                e = _q[_qi[0] % 3]
                _qi[0] += 1
                return e

            # out_p DMAs avoid the gpsimd queue: the causal-masking
            # affine_selects live there and sit on the exp->select->AV
            # critical path during attention
            _qo = [0]

            def dq_out():
                e = [nc.sync, nc.scalar][_qo[0] % 2]
                _qo[0] += 1
                return e

            # ---- persistent tiles ----
            kT_full = persist.tile([128, S], BF16, tag="kT_full")
            v_nat = persist.tile([128, NKS, 128], BF16, tag="v_nat")  # [ks, tile, d]
            ow_sb = persist.tile([128, GQ, H], BF16, tag="ow_sb")
            w_all = persist.tile([128, NT, KT, 128], BF16, tag="w_all")
            cos_full = persist.tile([128, S], BF16, tag="cos_full")
            ss_full = persist.tile([128, S], BF16, tag="ss_full")
            bias_sb = consts.tile([128, NT], F32, tag="bias_sb")

            q_chunks = {}

            def prefetch_x(c):
                if c >= NCH or c in x_tiles:
                    return
                x_c = p1s.tile([128, KT, CH], BF16, tag="x_c", name=f"x_{c}")
                for kq in range(4):
                    dq().dma_start(out=x_c[:, bass.ds(kq * 4, 4), :],
                                   in_=xT[c, :, bass.ds(kq * 4, 4), :])
                x_tiles[c] = x_c

            # ---- first-chunk-critical DMAs: just-in-time ordering ----
            # DMA issue order == transfer start order per queue, and the HBM
            # path is bandwidth-shared, so the pieces the first proj groups
            # consume must be issued FIRST; cos/ss (needed ~4us in) and ow
            # (needed ~40us in) go behind them.
            x_c0 = p1s.tile([128, KT, CH], BF16, tag="x_c", name="x_0")
            x_tiles = {0: x_c0}
            # deal the chunk-0 pieces to the 3 queues in CONSUMPTION order:
            # group t consumes all 16 x k-tiles plus its w halves; the k
            # group's second w half is needed ~1.7us in, so it must not sit
            # behind the whole x load. w halves (8 k-tiles) let each group
            # start after its first half lands; x starts with 1-kt pieces.
            def _xp(a, b):
                def f(eng):
                    eng.dma_start(out=x_c0[:, a:b, :], in_=xT[0, :, a:b, :])
                return f

            def _wp(t, k0):
                def f(eng):
                    eng.dma_start(out=w_all[:, t, k0:k0 + 8, :],
                                  in_=wT[:, t, k0:k0 + 8, :])
                return f

            _startup = [
                lambda eng: eng.dma_start(out=w_all[:, GQ, 0:4, :],
                                          in_=wT[:, GQ, 0:4, :]),
                _xp(0, 1), _xp(1, 2),
                lambda eng: eng.dma_start(out=w_all[:, GQ, 4:8, :],
                                          in_=wT[:, GQ, 4:8, :]),
                _xp(2, 4), _wp(GQ, 8), _xp(4, 6), _xp(6, 8),
                _wp(GQ + 1, 0), _xp(8, 10), _wp(GQ + 1, 8), _xp(10, 12),
                _wp(0, 0), _xp(12, 14), _wp(0, 8), _xp(14, 16),
                _wp(1, 0), _wp(1, 8), _wp(2, 0), _wp(2, 8),
                _wp(3, 0), _wp(3, 8),
            ]
            # bias is tiny but needed by the first evacuation (~3.5us in)
            _startup.insert(4, lambda eng: eng.dma_start(out=bias_sb,
                                                         in_=biasT[:, :]))
            for si, f in enumerate(_startup):
                f([nc.sync, nc.gpsimd, nc.scalar][si % 3])
            nc.scalar.dma_start(out=ss_full, in_=ssT[:, :])
            nc.gpsimd.dma_start(out=cos_full, in_=cosT[:, :])
            dummy_sb = consts.tile([1, 128], F32, tag="dummy_sb")
            nc.sync.dma_start(out=dummy_sb[:, 0:DL], in_=dummy[:, :])
            nc.gpsimd.dma_start(out=ow_sb, in_=owT[:, :, :])

            # warm the scalar-engine exp table (emitted after the critical
            # startup DMAs so its ACT_TABLE_LOAD doesn't block the scalar
            # queue; it only must precede attention(0)'s first real exp)
            warm = consts.tile([1, 2], F32, tag="warm")
            nc.vector.memset(warm, 0.0)
            warm2 = consts.tile([1, 2], F32, tag="warm2")
            nc.scalar.activation(out=warm2, in_=warm,
                                 func=mybir.ActivationFunctionType.Exp)

            # ---- small constants (engine ops, no DMA) ----
            ones_mat = consts.tile([128, 128], BF16, tag="ones_mat")
            nc.vector.memset(ones_mat, 1.0)
            ident = consts.tile([128, 128], BF16, tag="ident")
            make_identity(nc, ident)

            pending_oproj = [None]
            deferred_fin = []   # up to 2 of (finalize_fn, h, ps_o, acc); spans phases
            _evac = [0]
            _oq = [0]

            def make_oproj_items(args, final=False):
                """Return a list of closures, each emitting one o_proj matmul;
                the 4th of each group also emits the evacuation + out DMA."""
                cc, outT_ch = args
                items = []
                for st4 in range(CH // 128):
                    ssl = bass.ds(st4 * 128, 128)
                    dsl = bass.ds((cc * (CH // 128) + st4) * 128, 128)
                    for nch in range(NCH):
                        nsl = bass.ds(nch * CH, CH)

                        def mk(st4=st4, nch=nch, ssl=ssl, dsl=dsl, nsl=nsl):
                            state = {}

                            def first():
                                state["ps3"] = pp_p.tile([128, CH], F32, tag="p",
                                                         name="ps3")
                                nc.tensor.matmul(state["ps3"], outT_ch[0][:, ssl],
                                                 ow_sb[:, 0, nsl],
                                                 start=True, stop=False)

                            def mid(h):
                                def f():
                                    nc.tensor.matmul(state["ps3"],
                                                     outT_ch[h][:, ssl],
                                                     ow_sb[:, h, nsl],
                                                     start=False, stop=(h == GQ - 1))
                                    if h == GQ - 1:
                                        stg = stgp.tile([128, CH], BF16, tag="stg")
                                        if _evac[0] % 2 == 0:
                                            nc.scalar.activation(
                                                out=stg, in_=state["ps3"],
                                                func=mybir.ActivationFunctionType.Copy)
                                        else:
                                            nc.vector.tensor_copy(out=stg,
                                                                  in_=state["ps3"])
                                        _evac[0] += 1
                                        if final:
                                            # split across two queues: halves
                                            # the last tiles' transfer latency
                                            half = CH // 2
                                            n0 = nch * CH
                                            dq().dma_start(
                                                out=out_p[dsl, bass.ds(n0, half)],
                                                in_=stg[:, 0:half])
                                            dq().dma_start(
                                                out=out_p[dsl, bass.ds(n0 + half, half)],
                                                in_=stg[:, half:CH])
                                        else:
                                            dq_out().dma_start(
                                                out=out_p[dsl, nsl], in_=stg)
                                return f

                            return [first] + [mid(h) for h in range(1, GQ)]

                        items.extend(mk())
                return items

            def build_proj_groups(c):
                """Projection matmuls for chunk c as one closure per weight
                group, ordered [k, q0, v, q1, q2, q3]: the first three run
                before attention(c); q1..q3 weave into the attention head loop
                (head h's exp time covers head h+1's projection group)."""
                sl = bass.ds(c * CH, CH)
                x_c = x_tiles[c]
                q_ch = [qch_pool.tile([128, CH], BF16, tag=f"qch{h}",
                                      name=f"qch{h}_{c}")
                        for h in range(GQ)]
                q_chunks[c] = q_ch

                def mk_group(t):
                    def f():
                        ps_p = pp_p.tile([128, CH], F32, tag="p", name="ps_p")
                        for kt in range(KT):
                            nc.tensor.matmul(ps_p, w_all[:, t, kt, :],
                                             x_c[:, kt, :],
                                             start=(kt == 0), stop=(kt == KT - 1))
                        raw = p1.tile([128, CH], BF16, tag="raw",
                                      name=f"raw_{c}_{t}")
                        nc.scalar.activation(
                            out=raw, in_=ps_p,
                            func=mybir.ActivationFunctionType.Identity,
                            bias=bias_sb[:, bass.ds(t, 1)])
                        if t == NT - 1:   # v: no rope; transpose to v_nat
                            for i4 in range(CH // 128):
                                i = c * (CH // 128) + i4
                                ps_t = pp_p.tile([128, 128], BF16, tag="p",
                                                 name="ps_t")
                                nc.tensor.transpose(
                                    ps_t, raw[:, bass.ds(i4 * 128, 128)], ident)
                                nc.vector.tensor_copy(out=v_nat[:, i, :],
                                                      in_=ps_t)
                        else:
                            # rope: dst = raw*cos + swap(raw)*ss
                            sw = p1.tile([128, CH], BF16, tag="sw",
                                         name=f"sw_{c}_{t}")
                            dq().dma_start(out=sw[0:64, :], in_=raw[64:128, :])
                            dq().dma_start(out=sw[64:128, :], in_=raw[0:64, :])
                            nc.vector.tensor_mul(sw, sw, ss_full[:, sl])
                            dst = q_ch[t] if t < GQ else kT_full[:, sl]
                            nc.vector.tensor_mul(dst, raw, cos_full[:, sl])
                            nc.vector.tensor_add(dst, dst, sw)
                    return f

                return [mk_group(t) for t in [GQ, GQ + 1] + list(range(GQ))]

            def emit_attention(j, oproj_items, qgroups=()):
                """Attention for qs-chunk j; interleaves pending o_proj matmul
                items (from the previous chunk) into the pair loop."""
                sl = bass.ds(j * CH, CH)
                live = live_per_j[j]
                masked = set(masked_per_j[j])
                # pair up live tiles; odd count -> trailing singleton
                pairs = [(live[2 * m], live[2 * m + 1] if 2 * m + 1 < len(live)
                          else None) for m in range((len(live) + 1) // 2)]
                npairs = len(pairs)
                oi = [0]
                total_slots = GQ * (npairs + 1)
                slot = [0]

                def pull_oproj():
                    if not oproj_items:
                        return
                    remaining = len(oproj_items) - oi[0]
                    slots_left = total_slots - slot[0]
                    k = -(-remaining // max(1, slots_left))  # ceil
                    for _ in range(k):
                        if oi[0] < len(oproj_items):
                            oproj_items[oi[0]]()
                            oi[0] += 1
                    slot[0] += 1

                outT_ch = [outp_pool.tile([128, CH], BF16, tag=f"outT{h}",
                                          name=f"outT{h}_{j}") for h in range(GQ)]
                q_ch = q_chunks[j]

                def finalize(h, ps_o, acc):
                    # ones[128,128] stationary: column sums broadcast to all
                    # partitions in one matmul; then fast reciprocal + multiply.
                    ps_den = pp_den.tile([128, CH], F32, tag="den", name="ps_den")
                    nc.tensor.matmul(ps_den, ones_mat, acc, start=True, stop=True)
                    recip = fin.tile([128, CH], F32, tag="recip")
                    nc.vector.reciprocal_approx_fast(out=recip, in_=ps_den)
                    nc.vector.tensor_mul(outT_ch[h], ps_o, recip)

                for h in range(GQ):
                    qh = q_ch[h]
                    acc = accp.tile([128, CH], BF16, tag="acc", name=f"acc_{j}_{h}")
                    n_mm = sum(1 if i1 is None else 2 for i0, i1 in pairs)
                    attns = {}

                    def off_of(i):
                        # causal diagonal tile at offset d = i-4j: columns
                        # q < 128*d are fully masked -> clip them everywhere
                        if causal and i is not None and i in masked:
                            return 128 * (i - 4 * j)
                        return 0

                    def emit_qk_exp(n, h=h, qh=qh):
                        i0, i1 = pairs[n]
                        o0, o1 = off_of(i0), off_of(i1)
                        qk = pp_qk.tile([128, 2, CH], F32, tag="qk", name="qk")
                        nc.tensor.matmul(qk[:, 0, o0:],
                                         kT_full[:, bass.ds(i0 * 128, 128)],
                                         qh[:, o0:], start=True, stop=True)
                        if i1 is not None:
                            nc.tensor.matmul(qk[:, 1, o1:],
                                             kT_full[:, bass.ds(i1 * 128, 128)],
                                             qh[:, o1:], start=True, stop=True)
                        attn = attnp.tile([128, 2, CH], BF16, tag="attn",
                                          name="attn")
                        if i1 is not None and o0 == 0 and o1 <= 128:
                            # one paired exp is cheaper than two split ones for
                            # small clips; the clipped columns get exp(stale)
                            # which nothing consumes
                            nc.scalar.activation(out=attn, in_=qk,
                                                 func=mybir.ActivationFunctionType.Exp,
                                                 scale=float(ATTN_SCALE))
                        else:
                            for k, (i, o) in enumerate(((i0, o0), (i1, o1))):
                                if i is None:
                                    continue
                                nc.scalar.activation(out=attn[:, k, o:],
                                                     in_=qk[:, k, o:],
                                                     func=mybir.ActivationFunctionType.Exp,
                                                     scale=float(ATTN_SCALE))
                        for k, (i, o) in enumerate(((i0, o0), (i1, o1))):
                            if i is None or i not in masked:
                                continue
                            if causal:
                                # zero the partially-masked triangle: keep
                                # where q' >= p (q' relative to the clipped
                                # slice start 128*d); is_gt is the only ALU op
                                # the compiler implements for affine_select
                                nc.gpsimd.affine_select(
                                    out=attn[:, k, o:], in_=attn[:, k, o:],
                                    compare_op=mybir.AluOpType.is_gt,
                                    fill=0.0,
                                    base=1,
                                    channel_multiplier=-1,
                                    pattern=[[1, CH - o]],
                                )
                            else:
                                mt = p2.tile([128, CH], BF16, tag="m_tile",
                                             name=f"mt_{j}_{h}_{i}")
                                dq().dma_start(out=mt,
                                               in_=emaskT[bass.ds(i * 128, 128), sl])
                                nc.vector.tensor_mul(attn[:, k, :], attn[:, k, :], mt)
                        attns[n] = attn

                    # lookahead: 2 score-pair tiles in flight before the first
                    # AV; the previous head's finalize (PE ps_den matmul) must
                    # be emitted BEFORE this head's first AV (ps_o bufs=1).
                    emit_qk_exp(0)
                    if npairs > 1:
                        emit_qk_exp(1)
                    if h < len(qgroups):
                        # next head's projection group: its PE time is covered
                        # by this head's exp backlog on the scalar engine
                        qgroups[h]()
                    # flush deferred finalizes BEFORE pulling o_proj items:
                    # at a phase boundary those items read the outT tiles the
                    # finalizes write (emission order defines the dependency).
                    # ps_o has 2 banks, so each finalize can ride 2 heads
                    # behind its accumulation — the den matmul then never
                    # waits on the DVE chain.
                    while deferred_fin:
                        fn, fh, fo, fa = deferred_fin.pop(0)
                        fn(fh, fo, fa)
                    pull_oproj()
                    ps_o = pp_o.tile([128, CH], F32, tag="o", name="ps_o")
                    mm_done = 0
                    for n in range(npairs):
                        if n + 2 < npairs:
                            emit_qk_exp(n + 2)
                        i0, i1 = pairs[n]
                        o0, o1 = off_of(i0), off_of(i1)
                        attn = attns.pop(n)
                        # denominator accumulation on DVE (bf16, 2x mode).
                        # clipped (fully-masked) columns contribute zero and
                        # are skipped; the first live tile of any chunk always
                        # covers the full width, so acc is fully initialized.
                        if n == 0:
                            assert o0 == 0
                            if i1 is None:
                                nc.vector.tensor_copy(out=acc, in_=attn[:, 0, :])
                            elif o1 == 0:
                                nc.vector.tensor_add(acc, attn[:, 0, :], attn[:, 1, :])
                            else:
                                nc.vector.tensor_copy(out=acc, in_=attn[:, 0, :])
                                nc.vector.tensor_add(acc[:, o1:], acc[:, o1:],
                                                     attn[:, 1, o1:])
                        elif i1 is not None and o0 == 0 and o1 == 0:
                            tmp = tmpp.tile([128, CH], BF16, tag="tmp")
                            nc.vector.tensor_add(tmp, attn[:, 0, :], attn[:, 1, :])
                            nc.vector.tensor_add(acc, acc, tmp)
                        else:
                            for k, (i, o) in enumerate(((i0, o0), (i1, o1))):
                                if i is None:
                                    continue
                                nc.vector.tensor_add(acc[:, o:], acc[:, o:],
                                                     attn[:, k, o:])
                        # AV accumulation (clipped to live columns)
                        for k, (i, o) in enumerate(((i0, o0), (i1, o1))):
                            if i is None:
                                continue
                            nc.tensor.matmul(ps_o[:, o:], v_nat[:, i, :],
                                             attn[:, k, o:],
                                             start=(mm_done == 0),
                                             stop=(mm_done == n_mm - 1))
                            mm_done += 1
                        pull_oproj()
                    # the trailing finalizes deliberately span into the NEXT
                    # phase: their den-matmuls would otherwise stall the PE
                    # queue on the DVE accumulation chain at the boundary
                    deferred_fin.append((finalize, h, ps_o, acc))
                # drain any leftover o_proj items
                while oproj_items and oi[0] < len(oproj_items):
                    oproj_items[oi[0]]()
                    oi[0] += 1
                return outT_ch

            projected = set()

            def ensure_proj(c):
                if c >= NCH or c in projected:
                    return
                projected.add(c)
                prefetch_x(c)
                for g in build_proj_groups(c):
                    g()

            # NOTE: weaving projection matmuls into the attention pair loop
            # (tried at both item and group granularity) consistently REGRESSED
            # ~10-40us: the tighter cross-engine coupling costs ~60ns of
            # semaphore wait per matmul, exceeding the overlap gain. Keep the
            # projection phases sequential; only o_proj interleaves.
            built = {}

            def groups_for(c):
                if c not in built:
                    prefetch_x(c)
                    built[c] = build_proj_groups(c)
                return built[c]

            for c in range(NCH):
                ensure_proj(c)
                for j in range(NCH):
                    if need[j] == c:
                        oproj_items = (make_oproj_items(pending_oproj[0])
                                       if pending_oproj[0] is not None else [])
                        prefetch_x(c + 1)
                        outT = emit_attention(j, oproj_items)
                        pending_oproj[0] = (j, outT)

            while deferred_fin:
                fn, fh, fo, fa = deferred_fin.pop(0)
                fn(fh, fo, fa)
            if pending_oproj[0] is not None:
                for it in make_oproj_items(pending_oproj[0], final=True):
                    it()

    nc.finalize()
    return nc


_cache = {}


def _get_program(key, cls_grid, causal):
    if key not in _cache:
        _cache[key] = _build(cls_grid, causal)
    return _cache[key]


def _classify(em_t):
    """em_t: exp(mask).T [S, S] (ks, qs). Returns tuple-of-tuples class grid
    [NKS][NCH]."""
    grid = []
    for i in range(NKS):
        row = []
        for j in range(NCH):
            t = em_t[i * 128:(i + 1) * 128, j * CH:(j + 1) * CH]
            mx = t.max()
            mn = t.min()
            if mx == 0.0:
                row.append(SKIP)
            elif mn == 1.0 and mx == 1.0:
                row.append(PLAIN)
            else:
                row.append(MASKED)
        grid.append(tuple(row))
    return tuple(grid)


def _causal_grid():
    g = []
    for i in range(NKS):
        row = []
        for j in range(NCH):
            if i >= 4 * j + 4:
                row.append(SKIP)
            elif i >= 4 * j:
                row.append(MASKED)
            else:
                row.append(PLAIN)
        g.append(tuple(row))
    return tuple(g)


def _is_exact_causal(emaskT_b):
    """True iff exp(mask).T's diagonal band is exactly the causal 0/1
    pattern (off-band is covered by the grid comparison)."""
    p = np.arange(128)[:, None]
    for jj in range(NCH):
        for i in range(4 * jj, 4 * jj + 4):
            t = emaskT_b[i * 128:(i + 1) * 128, jj * CH:(jj + 1) * CH]
            d = i - 4 * jj
            q = np.arange(CH)[None, :]
            want = (p - q + 128 * d <= 0).astype(np.float32)
            if not np.array_equal(t, want):
                return False
    return True


def kernel(hidden_states, cos, sin, attention_mask,
           q_w, k_w, v_w, q_b, k_b, v_b,
           q_A, q_B, k_A, k_B, v_A, v_B, o_w):
    f32 = np.float32
    hidden_states = np.ascontiguousarray(hidden_states, dtype=f32)
    cos = np.asarray(cos, dtype=f32)
    sin = np.asarray(sin, dtype=f32)
    mask = np.asarray(attention_mask, dtype=f32)[:, 0]  # [B, S, S]

    # host-side shared prep
    with np.errstate(under="ignore", over="ignore"):
        emask = np.exp(np.minimum(mask, 80.0))  # [B, S, S]; clamp avoids inf
    emaskT = [np.ascontiguousarray(emask[b].T) for b in range(B)]
    grids = [_classify(emaskT[b]) for b in range(B)]
    if grids[0] != grids[1]:
        # classifications must agree across cores (same SPMD program):
        # degrade to "multiply everywhere except both-skip"
        grid = tuple(tuple(MASKED if (grids[0][i][j] != SKIP or grids[1][i][j] != SKIP)
                           else SKIP for j in range(NCH)) for i in range(NKS))
    else:
        grid = grids[0]
    # every qs column needs at least one live tile (else div by zero);
    # fall back to fully dense+masked if any column is empty
    for j in range(NCH):
        if all(grid[i][j] == SKIP for i in range(NKS)):
            grid = tuple(tuple(MASKED for _ in range(NCH)) for _ in range(NKS))
            break

    causal = (grid == _causal_grid()
              and all(_is_exact_causal(emaskT[b]) for b in range(B)))

    nc = _get_program((grid, causal), grid, causal)

    # x_pre[c, p, kt, s'] = x[b][c*CH+s', kt*128+p]
    xT = [np.ascontiguousarray(
        hidden_states[b].reshape(NCH, CH, KT, 128).transpose(0, 3, 2, 1)).astype(BF16_NP)
        for b in range(B)]
    cosT = [np.ascontiguousarray(cos[b].T).astype(BF16_NP) for b in range(B)]
    ss = np.concatenate([-sin[:, :, :HD // 2], sin[:, :, HD // 2:]], axis=-1)  # [B,S,HD]
    ssT = [np.ascontiguousarray(ss[b].T).astype(BF16_NP) for b in range(B)]

    # fold LoRA into the base weights (exact same math, done in fp32 on host)
    q_lora = LORA_SCALE * (q_A @ q_B).T   # [q_dim, H]
    k_lora = LORA_SCALE * (k_A @ k_B).T   # [kv_dim, H]
    v_lora = LORA_SCALE * (v_A @ v_B).T

    in_maps = []
    for c in range(NCORES):
        b, g = divmod(c, KVH)
        qsl = slice(QD * g, QD * (g + 1))
        ksl = slice(HD * g, HD * (g + 1))
        w_cat = np.concatenate([q_w[qsl] + q_lora[qsl],
                                k_w[ksl] + k_lora[ksl],
                                v_w[ksl] + v_lora[ksl]], axis=0)  # [768, H]
        # w_pre[p, t, kt, o] = w_cat[t*128+o, kt*128+p]
        wT_c = w_cat.reshape(NT, 128, KT, 128).transpose(3, 0, 2, 1)
        bias_c = np.concatenate([q_b[qsl], k_b[ksl], v_b[ksl]])  # [768]
        biasT_c = np.ascontiguousarray(bias_c.reshape(NT, 128).T, dtype=f32)
        owT_c = np.ascontiguousarray(
            o_w[:, qsl].T.reshape(GQ, 128, H).transpose(1, 0, 2)).astype(BF16_NP)
        m = {
            "xT": xT[b],
            "wT": np.ascontiguousarray(wT_c).astype(BF16_NP),
            "biasT": biasT_c,
            "cachetag": np.zeros((1, (K_TAG_INT % 97) + 1), f32),
            "cosT": cosT[b],
            "ssT": ssT[b],
            "owT": owT_c,
        }
        if not causal and any(grid[i][j] == MASKED for i in range(NKS) for j in range(NCH)):
            m["emaskT"] = emaskT[b].astype(BF16_NP)
        in_maps.append(m)

    res = run_bass_kernel_spmd(nc, in_maps, core_ids=list(range(NCORES)))
    outs = [np.asarray(r["out_p"]).astype(f32) for r in res.results]
    full = np.empty((B, S, H), f32)
    for b in range(B):
        full[b] = outs[KVH * b]
        for g in range(1, KVH):
            full[b] += outs[KVH * b + g]
    return full
